# revision 1
# baseline (speedup 1.0000x reference)
"""Trainium2 kernel for nn_KerasDense_32263794328408.

y = relu(x @ M + b), where M is a 4096x4096 TT-matrix (cores of shape
[r_{k-1}, 8, 8, r_k], ranks [1,8,8,8,1]).

Strategy: the TT cores are tiny (<17 KB each); materialize the dense
M = 4096x4096 on the host (cheap, ~270 MFLOP) and run the dense
y = relu(x @ M + b) as a near-roofline GEMM on 8 NeuronCores.

Sharding: 2D grid, 4 batch groups x 2 output-column groups.
Per core: x-shard [1024, 4096] (shipped transposed as xT [4096, 1024]),
W column-half [4096, 2048] and bias half, producing y [1024, 2048].

Inputs are shipped in bf16 (quantization absmax/scale ~2.5e-3, well
under the 2e-2 gate). vs fp32r this halves HBM traffic — the f32
version was pinned at the per-core HBM ceiling during the x-load phase
and stalled the PE — and bf16 LDWEIGHTS gets fast-weight-load, so the
stationary reload hides fully under the matmul stream.

On-chip: x-stationary matmuls. lhsT = xT tile [128k x 128b], rhs = W
slab [128k x 512o], PSUM accumulation over the 32 k-tiles plus a K=1
matmul adding the bias. W is host-pre-blocked so every slab DMA is a
fully contiguous [128, 2048] read (4 KB per partition line). The last
k-quad of each output chunk runs bt-major with the bias matmul and the
DVE relu+store chained per batch tile, so PSUM banks free one by one
and the drains overlap the next chunk's matmuls (short kernel tail).
"""

import sys

if "/opt/trn_rl_repo" not in sys.path:
    sys.path.insert(0, "/opt/trn_rl_repo")

import ml_dtypes
import numpy as np

import concourse.bacc as bacc
import concourse.bass as bass
import concourse.mybir as mybir
import concourse.tile as tile
from concourse.bass_utils import run_bass_kernel_spmd

F32 = mybir.dt.float32
BF16 = mybir.dt.bfloat16
F8E4 = mybir.dt.float8e4
NP_BF16 = ml_dtypes.bfloat16
NP_F8 = mybir.dt.np(F8E4)

# One k-tile pair computed as a single fp8-e4m3 DoubleRow matmul (contracts
# K=256 in one pass at ~2x rate). Quantizing 2 of 32 k-tiles to fp8 raises
# absmax/scale from ~3.9e-3 to ~1.1e-2 (gate 2e-2) and saves ~6us of PE time.
KP = 24  # pair covers kt 24 and 25 (ktq 6, k4 0..1)

B_FULL = 4096  # batch
F_FULL = 4096  # input features
O_FULL = 4096  # output features

BG = 4  # batch groups
OG = 2  # output-column groups
N_CORES = BG * OG

B_L = B_FULL // BG   # 1024 batch rows per core
O_L = O_FULL // OG   # 2048 output cols per core
KT = F_FULL // 128   # 32 contraction tiles
OC = O_L // 512      # 4 output chunks of 512 per core
BT = B_L // 128      # 8 batch tiles of 128 per core
KQ = 4               # k-tiles fetched per W DMA
NQ = KT // KQ        # 8 quad fetches per oc

_CACHE: dict = {}


def _build_module() -> bass.Bass:
    nc = bacc.Bacc(None, target_bir_lowering=False)

    xT = nc.declare_dram_parameter("xT", [F_FULL, B_L], BF16, isOutput=False)
    # w is host-pre-blocked: row si*128+p, col k4*512+c holds
    # W[(ktq*4+k4)*128 + p, oc*512 + c] with si = oc*NQ + ktq, so each
    # slab (oc, ktq) is one contiguous [128, 2048] read.
    w = nc.declare_dram_parameter("w", [F_FULL, O_L], BF16, isOutput=False)
    bvec = nc.declare_dram_parameter("bvec", [1, O_L], BF16, isOutput=False)
    ones = nc.declare_dram_parameter("ones", [128, 128], BF16, isOutput=False)
    bbc = nc.declare_dram_parameter("bbc", [128, O_L], F32, isOutput=False)
    # fp8 DoubleRow pair operands: x8[p, i*B_L + b] = e4m3(xT[(KP+i)*128+p, b]),
    # w8[oc*128 + p, i*512 + c] = e4m3(W[(KP+i)*128+p, oc*512+c]).
    x8 = nc.declare_dram_parameter("x8", [128, 2 * B_L], F8E4, isOutput=False)
    w8 = nc.declare_dram_parameter("w8", [OC * 128, 1024], F8E4, isOutput=False)
    # y ships back as bf16 (host upcasts): halves store traffic and the
    # tail store; costs ~2e-3 extra absmax, still 5x under the gate.
    y = nc.declare_dram_parameter("y", [B_L, O_L], BF16, isOutput=True)

    with tile.TileContext(nc) as tc:
        with (
            tc.tile_pool(name="xt", bufs=1) as xt_pool,
            tc.tile_pool(name="w0", bufs=1) as w0_pool,
            tc.tile_pool(name="wsl", bufs=6) as w_pool,
            tc.tile_pool(name="yst", bufs=12) as y_pool,
            tc.tile_pool(name="cst", bufs=1) as c_pool,
            tc.tile_pool(name="acc", bufs=8, space="PSUM") as psum_pool,
        ):
            # Tiny constants first so they land with queue startup (the ones
            # tile is kept small — everything ahead of the x tiles on this
            # queue delays the cold-start x stream).
            ones_sb = c_pool.tile([128, 128], BF16, tag="ones")
            nc.scalar.dma_start(out=ones_sb[:], in_=ones[:])
            bias_sb = c_pool.tile([1, O_L], BF16, tag="bias")
            nc.scalar.dma_start(out=bias_sb[:], in_=bvec[:])

            # The first two W quads as separate [128, 512] tiles so early
            # matmuls only wait on 128 KB loads, and the first few x tiles
            # riding BOTH rings, interleaved on the SP ring in consumption
            # order (kt0 pairs with W chunk k4=0, kt1 with k4=1, ...) — the
            # cold-start window is paced entirely by these arrivals.
            def _w0_tile(ktq, k4):
                t = w0_pool.tile([128, 512], BF16, tag=f"w0_{ktq}_{k4}",
                                 name=f"w0_{ktq}_{k4}")
                nc.sync.dma_start(
                    out=t[:],
                    in_=w[ktq * 128 : (ktq + 1) * 128,
                          k4 * 512 : (k4 + 1) * 512],
                )
                return t

            xts = [None] * KT

            def _xt_tile(kt, eng):
                t = xt_pool.tile([128, B_L], BF16, tag=f"xt{kt}", name=f"xt{kt}")
                eng.dma_start(out=t[:], in_=xT[kt * 128 : (kt + 1) * 128, :])
                xts[kt] = t

            w0q = {}
            w0q[(0, 0)] = _w0_tile(0, 0)
            _xt_tile(0, nc.scalar)
            _xt_tile(1, nc.sync)
            w0q[(0, 1)] = _w0_tile(0, 1)
            _xt_tile(2, nc.scalar)
            _xt_tile(3, nc.sync)
            w0q[(0, 2)] = _w0_tile(0, 2)
            _xt_tile(4, nc.scalar)
            _xt_tile(5, nc.sync)
            w0q[(0, 3)] = _w0_tile(0, 3)
            for k4 in range(KQ):
                w0q[(1, k4)] = _w0_tile(1, k4)
            for kt in range(6, KT):
                _xt_tile(kt, nc.scalar)

            # fp8 pair stationary (first needed at ~55us, after the x tiles).
            x8_sb = c_pool.tile([128, 2 * B_L], F8E4, tag="x8")
            nc.scalar.dma_start(out=x8_sb[:], in_=x8[:])

            # Broadcast bias rows for the DVE drain; first needed when the
            # first output chunk finishes (~60 us), well after the x tiles
            # ahead of it on this ring.
            bbc_sb = c_pool.tile([128, O_L], F32, tag="bbc")
            nc.scalar.dma_start(out=bbc_sb[:], in_=bbc[:])

            # Warm-up matmuls on the tiny bias constant: the PE HAM clock
            # gate needs ~3.4 us of sustained activity to lift the cold
            # 1.2 GHz throttle, and the first real matmul can't start until
            # its x/W tiles land (~12 us: NEFF preamble + first loads).
            # Burning that idle window on dummy K=1 matmuls makes the real
            # stream run warm from its first instruction.
            # Full-K warmups: K=1 matmuls light only 1 of 128 PE rows and
            # barely register with the HAM activity monitor (flip came ~10us
            # after they started); full 128-row matmuls flip it on schedule.
            # 32 x N=128 covers the ~3.4us activity window.
            warm = psum_pool.tile([128, 512], F32, tag="acc", name="warm")
            for i in range(32):
                nc.tensor.matmul(
                    warm[:, 0:128],
                    ones_sb[:],
                    ones_sb[:],
                    start=(i == 0),
                    stop=(i == 31),
                )

            for oc in range(OC):
                accs = [
                    psum_pool.tile([128, 512], F32, tag="acc",
                                   name=f"acc_{oc}_{bt}")
                    for bt in range(BT)
                ]
                for ktq in range(NQ):
                    si = oc * NQ + ktq
                    if oc == 0 and ktq < 2:
                        wchunks = [w0q[(ktq, k4)][:] for k4 in range(KQ)]
                    else:
                        w_sl = w_pool.tile([128, KQ * 512], BF16, tag="wsl",
                                           name=f"w_{oc}_{ktq}")
                        nc.sync.dma_start(
                            out=w_sl[:], in_=w[si * 128 : (si + 1) * 128, :]
                        )
                        wchunks = [
                            w_sl[:, k4 * 512 : (k4 + 1) * 512] for k4 in range(KQ)
                        ]
                    if ktq == KP // KQ:
                        # kt 24+25 as one fp8 DoubleRow matmul per bt
                        # (3D APs [p, 2, free]; out = sum of both k-planes),
                        # then kt 26,27 in bf16 from the regular slab.
                        w8_sb = w_pool.tile([128, 1024], F8E4, tag="w8",
                                            name=f"w8_{oc}")
                        nc.sync.dma_start(
                            out=w8_sb[:],
                            in_=w8[oc * 128 : (oc + 1) * 128, :],
                        )
                        x8_3d = x8_sb[:].rearrange("p (i b) -> p i b", i=2)
                        w8_3d = w8_sb[:].rearrange("p (i c) -> p i c", i=2)
                        for bt in range(BT):
                            nc.tensor.matmul(
                                accs[bt][:],
                                x8_3d[:, :, bt * 128 : (bt + 1) * 128],
                                w8_3d,
                                start=False,
                                stop=False,
                                perf_mode=mybir.MatmulPerfMode.DoubleRow,
                            )
                        for k4 in (2, 3):
                            kt = ktq * KQ + k4
                            for bt in range(BT):
                                nc.tensor.matmul(
                                    accs[bt][:],
                                    xts[kt][:, bt * 128 : (bt + 1) * 128],
                                    wchunks[k4],
                                    start=False,
                                    stop=False,
                                )
                    elif ktq < NQ - 1:
                        for k4 in range(KQ):
                            kt = ktq * KQ + k4
                            for bt in range(BT):
                                nc.tensor.matmul(
                                    accs[bt][:],
                                    xts[kt][:, bt * 128 : (bt + 1) * 128],
                                    wchunks[k4],
                                    start=(kt == 0),
                                    stop=False,
                                )
                    else:
                        # Last quad bt-major: each batch tile finishes its
                        # contraction and drains while the PE moves on —
                        # banks free one at a time, drains and stores
                        # overlap the next oc's matmuls.
                        #
                        # Mid-kernel, bias + relu happen on the (otherwise
                        # idle) DVE instead of costing PE matmuls, and the
                        # y stores stay OFF the sync queue: HWDGE queues
                        # are strictly in-order, so a store parked behind
                        # an unmet semaphore would block the next chunk's
                        # W-slab fetch behind it and stall the PE.
                        #
                        # For the final chunk there are no W fetches left
                        # to protect, but the drain chain IS the kernel
                        # tail — so spend 8 K=1 matmuls on the bias to keep
                        # the DVE drain single-op (0.7us < 0.86us matmul
                        # spacing) and split the stores across both queues.
                        last_oc = oc == OC - 1
                        y_sls = []
                        for bt in range(BT):
                            for k4 in range(KQ):
                                kt = ktq * KQ + k4
                                nc.tensor.matmul(
                                    accs[bt][:],
                                    xts[kt][:, bt * 128 : (bt + 1) * 128],
                                    wchunks[k4],
                                    start=False,
                                    stop=(not last_oc and kt == KT - 1),
                                )
                            y_sl = y_pool.tile([128, 512], BF16, tag="yst",
                                               name=f"y_{oc}_{bt}")
                            y_sls.append(y_sl)
                            if last_oc:
                                nc.tensor.matmul(
                                    accs[bt][:],
                                    ones_sb[0:1, 0:128],
                                    bias_sb[:, oc * 512 : (oc + 1) * 512],
                                    start=False,
                                    stop=True,
                                )
                                if bt < BT - 1:
                                    nc.vector.tensor_scalar_max(
                                        y_sl[:], accs[bt][:], 0.0
                                    )
                                    dma_eng = (
                                        nc.scalar if bt % 2 == 0 else nc.sync
                                    )
                                    dma_eng.dma_start(
                                        out=y[
                                            bt * 128 : (bt + 1) * 128,
                                            oc * 512 : (oc + 1) * 512,
                                        ],
                                        in_=y_sl[:],
                                    )
                                else:
                                    # Very last tile: halve the relu+store
                                    # and fan across both queues — this
                                    # chain IS the kernel tail.
                                    for h, dma_eng in enumerate(
                                        (nc.sync, nc.scalar)
                                    ):
                                        cols = slice(h * 256, (h + 1) * 256)
                                        nc.vector.tensor_scalar_max(
                                            y_sl[:, cols], accs[bt][:, cols], 0.0
                                        )
                                        dma_eng.dma_start(
                                            out=y[
                                                bt * 128 : (bt + 1) * 128,
                                                oc * 512 + h * 256 : oc * 512
                                                + (h + 1) * 256,
                                            ],
                                            in_=y_sl[:, cols],
                                        )
                            else:
                                # Bias-add on DVE frees the PSUM bank in
                                # 0.7us — under the 0.86us/bt matmul pace,
                                # so the next chunk's matmuls never wait on
                                # a bank. Only the adds sit on the DVE
                                # during the handoff; relu + store follow
                                # below once all banks are clear.
                                nc.vector.tensor_add(
                                    y_sl[:],
                                    accs[bt][:],
                                    bbc_sb[:, oc * 512 : (oc + 1) * 512],
                                )
                        if not last_oc:
                            for bt in range(BT):
                                nc.vector.tensor_scalar_max(
                                    y_sls[bt][:], y_sls[bt][:], 0.0
                                )
                                nc.scalar.dma_start(
                                    out=y[
                                        bt * 128 : (bt + 1) * 128,
                                        oc * 512 : (oc + 1) * 512,
                                    ],
                                    in_=y_sls[bt][:],
                                )

    nc.finalize()
    return nc


def _materialize_w(core0, core1, core2, core3) -> np.ndarray:
    """Contract the TT cores into the dense 4096x4096 matrix M.

    M[(m1 m2 m3 m4), (n1 n2 n3 n4)] (big-endian mode order on both sides),
    matching the reference's x/y index conventions.
    """
    g1 = core0[0].astype(np.float64)            # [m1, n1, r1]
    t12 = np.einsum("mnr,rMNs->mMnNs", g1, core1.astype(np.float64))
    a12 = t12.reshape(64, 64, 8)                # [(m1 m2), (n1 n2), r2]
    g4 = core3[..., 0].astype(np.float64)       # [r3, m4, n4]
    t34 = np.einsum("rmns,sMN->rmMnN", core2.astype(np.float64), g4)
    b34 = t34.reshape(8, 64, 64)                # [r2, (m3 m4), (n3 n4)]
    w = np.einsum("mnr,rMN->mMnN", a12, b34)    # [(m12),(m34),(n12),(n34)]
    return np.ascontiguousarray(
        w.reshape(F_FULL, O_FULL), dtype=np.float32
    )


def _prepare_in_maps(x, w_full, bias):
    """Shard + bf16-quantize host-side. Core c = (g, h): batch group g,
    output-column group h."""
    xts = [
        np.ascontiguousarray(
            x[g * B_L : (g + 1) * B_L, :].T.astype(NP_BF16)
        )
        for g in range(BG)
    ]
    ws = []
    for h in range(OG):
        wh = w_full[:, h * O_L : (h + 1) * O_L].astype(NP_BF16)
        t = wh.reshape(NQ, KQ, 128, OC, 512)      # [ktq, k4, p, oc, c]
        wb = np.ascontiguousarray(t.transpose(3, 0, 2, 1, 4)).reshape(
            F_FULL, O_L
        )                                          # [(oc ktq p), (k4 c)]
        ws.append(wb)
    bs = [
        np.ascontiguousarray(bias[:, h * O_L : (h + 1) * O_L].astype(NP_BF16))
        for h in range(OG)
    ]
    bbcs = [
        np.ascontiguousarray(
            np.broadcast_to(bias[:, h * O_L : (h + 1) * O_L], (128, O_L))
        ).astype(np.float32)
        for h in range(OG)
    ]
    ones = np.ones((128, 128), dtype=NP_BF16)
    # fp8 DoubleRow pair operands for kt KP, KP+1.
    x8s = []
    for g in range(BG):
        xTg = x[g * B_L : (g + 1) * B_L, :].T  # [F_FULL, B_L] f32
        pair = np.stack(
            [xTg[(KP + i) * 128 : (KP + i + 1) * 128, :] for i in range(2)],
            axis=1,
        )  # [128, 2, B_L]
        x8s.append(np.ascontiguousarray(pair.reshape(128, 2 * B_L)).astype(NP_F8))
    w8s = []
    for h in range(OG):
        wh = w_full[:, h * O_L : (h + 1) * O_L]  # [F_FULL, O_L] f32
        pair = np.stack(
            [wh[(KP + i) * 128 : (KP + i + 1) * 128, :] for i in range(2)],
            axis=1,
        )  # [128, 2, O_L]
        blocks = [
            pair[:, :, oc * 512 : (oc + 1) * 512].reshape(128, 1024)
            for oc in range(OC)
        ]
        w8s.append(
            np.ascontiguousarray(np.concatenate(blocks, axis=0)).astype(NP_F8)
        )
    in_maps = []
    for c in range(N_CORES):
        g, h = divmod(c, OG)
        in_maps.append(
            {
                "xT": xts[g],
                "w": ws[h],
                "bvec": bs[h],
                "ones": ones,
                "bbc": bbcs[h],
                "x8": x8s[g],
                "w8": w8s[h],
            }
        )
    return in_maps


def kernel(x, core0, core1, core2, core3, b) -> np.ndarray:
    x = np.asarray(x, dtype=np.float32)
    w_full = _materialize_w(
        np.asarray(core0, dtype=np.float32),
        np.asarray(core1, dtype=np.float32),
        np.asarray(core2, dtype=np.float32),
        np.asarray(core3, dtype=np.float32),
    )
    bias = np.asarray(b, dtype=np.float32).reshape(1, O_FULL)

    if "nc" not in _CACHE:
        _CACHE["nc"] = _build_module()
    nc = _CACHE["nc"]

    in_maps = _prepare_in_maps(x, w_full, bias)
    res = run_bass_kernel_spmd(nc, in_maps, core_ids=list(range(N_CORES)))

    y = np.empty((B_FULL, O_FULL), dtype=np.float32)
    for c in range(N_CORES):
        g, h = divmod(c, OG)
        y[g * B_L : (g + 1) * B_L, h * O_L : (h + 1) * O_L] = np.asarray(
            res.results[c]["y"]
        ).astype(np.float32)
    return y



# revision 2
# speedup vs baseline: 1.0007x; 1.0007x over previous
"""Trainium2 TT-structured kernel for nn_KerasDense_32263794328408.

y = relu(x @ M + b), M = TT-matrix, ranks [1,8,8,8,1], modes 8x8x8x8.

Key algebra: merging cores (0,1) -> A[i12, j12, r] (64x64x8) and cores
(2,3) -> B[r, i34, j34] (8x64x64) gives M = sum_r A_r (x) B_r (rank-8
Kronecker sum). Contracting B first then A needs 34.4 GFLOP total vs
137.4 GFLOP dense -- 4x fewer.

Two-stage PE schedule per core (batch-sharded 8 ways, B_L=512):

Stage 1 (contract i34, K=64): z[b, i12, r, j34] = sum_i34 x[b,i12,i34]
  * B[r,i34,j34]. Stationary = x'' chunk [64 i34, 128 (b-hat,i12)]
  (one chunk = 2 batch rows x 64 i12), moving = Bmat [64, 512 (r,j34)].
  K=64 wastes half the array, so two chunks run CONCURRENTLY via
  tile_position row-tiling: even chunks in array rows 0-63, odd chunks
  in rows 64-127, separate PSUM banks. 256 chunks -> 128 pair-windows
  of 512 cycles -> full PE rate.

Stage 2 (contract (i12, r), K=512): y[b, j12, j34] = sum A z. r rides
  the accumulation loop (8 matmuls per output accumulating in PSUM),
  i12 rides the partitions: stationary At_r = diag(A_r, A_r)
  [128 (b-hat' i12), 128 (b-hat j12)] block-diagonal so the two batch
  rows sharing a chunk don't mix; moving = z1 chunk column slice
  [128, 64 j34]. 8 pairs share one PSUM bank ([128, 8*64]); bias is
  added with one K=64 matmul (stationary = duplicated identity,
  moving = bias rows) opening each bank's accumulation group.

z1 (16.8M elem/core) must cross PSUM->SBUF through the compute
engines (DMA has no PSUM port): drains are split DVE/ACT and overlap
the PE stream; ACT also does the relu+bf16 cast of each finished y
bank. y is stored bank-contiguous ([128, 512] blocks) and un-permuted
on the host so every store DMA is a fully-contiguous 128 KB write.

Everything ships bf16 (x, Bmat, At, bias); accumulation is fp32 in
PSUM. The extra z1 bf16 rounding adds ~0.06% error -- the absmax/scale
stays ~4e-3, well under the 2e-2 gate.
"""

import sys

if "/opt/trn_rl_repo" not in sys.path:
    sys.path.insert(0, "/opt/trn_rl_repo")

import ml_dtypes
import numpy as np

import concourse.bacc as bacc
import concourse.bass as bass
import concourse.mybir as mybir
import concourse.tile as tile
from concourse.bass_utils import run_bass_kernel_spmd

F32 = mybir.dt.float32
BF16 = mybir.dt.bfloat16
NP_BF16 = ml_dtypes.bfloat16

B_FULL = 4096
F_FULL = 4096
O_FULL = 4096
N_CORES = 8
B_L = B_FULL // N_CORES          # 512 batch rows per core

NPAIR = B_L // 2                 # 256 chunks (= batch pairs) per core
NBCH = 8                         # bchunks
PAIR_PER_BCH = NPAIR // NBCH     # 32
NWIN = PAIR_PER_BCH // 2         # 16 step1 windows per bchunk
NBANK = PAIR_PER_BCH // 8        # 4 step2 banks per bchunk
R = 8

_CACHE: dict = {}


def _build_module() -> bass.Bass:
    nc = bacc.Bacc(None, target_bir_lowering=False)

    # x'': row-tiled stationary chunks. Column block w = 128 cols holds
    # chunks 2w (partitions 0-63) and 2w+1 (partitions 64-127); within a
    # block, col m = bhat*64 + i12, partition (c%2)*64 + i34.
    xpp = nc.declare_dram_parameter("xpp", [128, NPAIR * 64], BF16, isOutput=False)
    # Bmat moving operand, both partition halves identical:
    # bmat[h*64 + i34, r*64 + j34] = B[r, i34, j34]
    bmat = nc.declare_dram_parameter("bmat", [128, 512], BF16, isOutput=False)
    # At_r block-diagonal stationaries, concat along free dim.
    atil = nc.declare_dram_parameter("atil", [128, R * 128], BF16, isOutput=False)
    # Bias: one K=64 matmul per bank. eye[j12p, bhat*64+j12]=delta,
    # brhs[j12p, s*64+j34] = bias[j12p*64+j34].
    eye = nc.declare_dram_parameter("eye", [128, 128], BF16, isOutput=False)
    brhs = nc.declare_dram_parameter("brhs", [128, 512], BF16, isOutput=False)
    # y stored bank-major: row g*128 + (bhat*64 + j12), col s*64 + j34;
    # host un-permutes. Keeps every store DMA fully contiguous.
    y = nc.declare_dram_parameter("y", [32 * 128, 512], BF16, isOutput=True)

    with tile.TileContext(nc) as tc:
        with (
            tc.tile_pool(name="xt", bufs=1) as x_pool,
            tc.tile_pool(name="cst", bufs=1) as c_pool,
            tc.tile_pool(name="z1", bufs=1) as z_pool,
            tc.tile_pool(name="ysb", bufs=1) as y_pool,
            tc.tile_pool(name="ps1", bufs=3, space="PSUM") as ps1_pool,
            tc.tile_pool(name="ps2", bufs=2, space="PSUM") as ps2_pool,
        ):
            # Warmup operand via memset: no DMA dependency, so the HAM
            # warmup matmuls start right after engine init (~4us) instead
            # of waiting for the first DMA (~10us).
            ones_sb = c_pool.tile([128, 128], BF16, tag="ones")
            nc.gpsimd.memset(ones_sb[:], 1.0)
            bmat_sb = c_pool.tile([128, 512], BF16, tag="bmat")
            nc.sync.dma_start(out=bmat_sb[:], in_=bmat[:])

            # x'' in 8 bchunk pieces; first two segments ahead of the
            # step2 constants (not needed until unit 2).
            xpp_sb = x_pool.tile([128, NPAIR * 64], BF16, tag="xpp")
            seg = PAIR_PER_BCH * 64
            sub = seg // 4
            for j in range(4):
                nc.sync.dma_start(
                    out=xpp_sb[:, j * sub : (j + 1) * sub],
                    in_=xpp[:, j * sub : (j + 1) * sub],
                )
            nc.sync.dma_start(
                out=xpp_sb[:, seg : 2 * seg], in_=xpp[:, seg : 2 * seg]
            )
            eye_sb = c_pool.tile([128, 128], BF16, tag="eye")
            nc.sync.dma_start(out=eye_sb[:], in_=eye[:])
            brhs_sb = c_pool.tile([128, 512], BF16, tag="brhs")
            nc.sync.dma_start(out=brhs_sb[:], in_=brhs[:])
            atil_sb = c_pool.tile([128, R * 128], BF16, tag="atil")
            nc.sync.dma_start(out=atil_sb[:], in_=atil[:])
            for k in range(2, NBCH):
                nc.sync.dma_start(
                    out=xpp_sb[:, k * seg : (k + 1) * seg],
                    in_=xpp[:, k * seg : (k + 1) * seg],
                )

            # HAM warmup: full-K dummy matmuls while the first loads land.
            warm = ps2_pool.tile([128, 512], F32, tag="ps2", name="warm")
            for i in range(40):
                nc.tensor.matmul(
                    warm[:, 0:128],
                    ones_sb[:],
                    ones_sb[:],
                    start=(i == 0),
                    stop=(i == 39),
                )

            ZB = 4  # z_bank tiles in flight (write g, drain g/g-1, read g-2)
            z_banks = [None] * ZB

            def step1_window(w):
                # window w: chunks 2w (rows 0-63), 2w+1 (rows 64-127),
                # both -> one [128, 1024] psum pair, one fused drain into
                # z_bank[w//4] cols (2w%8)*512 .. +1024.
                g = w // 4
                if w % 4 == 0:
                    zb = z_pool.tile(
                        [128, 8 * 512], BF16, tag=f"zb{g % ZB}",
                        name=f"zb_{g}"
                    )
                    z_banks[g % ZB] = zb
                zb = z_banks[g % ZB]
                ps = ps1_pool.tile([128, 1024], F32, tag="ps1",
                                   name=f"z_{w}")
                for half in (0, 1):
                    nc.tensor.matmul(
                        ps[:, half * 512 : half * 512 + 512],
                        xpp_sb[half * 64 : half * 64 + 64,
                               w * 128 : (w + 1) * 128],
                        bmat_sb[half * 64 : half * 64 + 64, :],
                        start=True,
                        stop=True,
                        tile_position=(half * 64, 0),
                    )
                dst = zb[:, (2 * w % 8) * 512 : (2 * w % 8) * 512 + 1024]
                if w % 2 == 0:
                    nc.scalar.copy(dst, ps[:])
                else:
                    nc.vector.tensor_copy(dst, ps[:])

            y_ps = {}

            def step2_piece(g, piece):
                # bank g's PE work, split into 5 pieces interleaved between
                # step1 windows: [bias], [r0 r1], [r2 r3], [r4 r5], [r6 r7
                # + relu + store].
                zb = z_banks[g % ZB]
                if piece == 0:
                    ps = ps2_pool.tile([128, 512], F32, tag="ps2",
                                       name=f"y_{g}")
                    y_ps[g % 2] = ps
                    nc.tensor.matmul(
                        ps[:], eye_sb[0:64, :], brhs_sb[0:64, :],
                        start=True, stop=False,
                        skip_group_check=True,
                    )
                    return
                ps = y_ps[g % 2]
                z3 = zb[:].rearrange("p (s q) -> p s q", q=512)
                for r in (2 * piece - 2, 2 * piece - 1):
                    nc.tensor.matmul(
                        ps[:],
                        atil_sb[:, r * 128 : (r + 1) * 128],
                        z3[:, :, r * 64 : (r + 1) * 64],
                        start=False,
                        stop=(r == R - 1),
                        skip_group_check=True,
                    )
                if piece == 4:
                    ysb = y_pool.tile(
                        [128, 512], BF16, tag=f"y{g % 4}", name=f"ysb_{g}"
                    )
                    if g % 3 != 0:
                        nc.scalar.activation(
                            ysb[:], ps[:], mybir.ActivationFunctionType.Relu
                        )
                    else:
                        nc.vector.tensor_scalar_max(ysb[:], ps[:], 0.0)
                    nc.sync.dma_start(
                        out=y[g * 128 : (g + 1) * 128, :], in_=ysb[:]
                    )

            NG = NPAIR // 8  # 32 banks
            # Unit u: windows 4u..4u+3 + step2 of bank u-2, grouped to
            # minimize PE shape transitions (each win<->step2 switch costs
            # ~100ns of exposed LDWEIGHTS): [winA winB bias][r0-r3]
            # [winC winD][r4-r7 relu].
            for u in range(NG + 2):
                if u < NG:
                    step1_window(4 * u)
                    step1_window(4 * u + 1)
                if u >= 2:
                    step2_piece(u - 2, 0)   # bias
                    step2_piece(u - 2, 1)   # r0 r1
                    step2_piece(u - 2, 2)   # r2 r3
                if u < NG:
                    step1_window(4 * u + 2)
                    step1_window(4 * u + 3)
                if u >= 2:
                    step2_piece(u - 2, 3)   # r4 r5
                    step2_piece(u - 2, 4)   # r6 r7 + relu + store

    nc.finalize()
    return nc


def _merge_cores(core0, core1, core2, core3):
    g1 = core0[0].astype(np.float64)            # [m1, n1, r1]
    t12 = np.einsum("mnr,rMNs->mMnNs", g1, core1.astype(np.float64))
    A = t12.reshape(64, 64, 8)                  # [i12, j12, r]
    g4 = core3[..., 0].astype(np.float64)       # [r3, m4, n4]
    t34 = np.einsum("rmns,sMN->rmMnN", core2.astype(np.float64), g4)
    B = t34.reshape(8, 64, 64)                  # [r, i34, j34]
    return A.astype(np.float32), B.astype(np.float32)


def _prepare_in_maps(x, A, B, bias):
    # Shared constants
    bm = np.ascontiguousarray(
        np.tile(B.transpose(1, 0, 2).reshape(64, 512), (2, 1))
    ).astype(NP_BF16)                            # [128, 512]
    at = np.zeros((R, 128, 128), dtype=np.float32)
    for r in range(R):
        at[r, 0:64, 0:64] = A[:, :, r]
        at[r, 64:128, 64:128] = A[:, :, r]
    at = np.ascontiguousarray(
        at.transpose(1, 0, 2).reshape(128, R * 128)
    ).astype(NP_BF16)
    eyev = np.ascontiguousarray(
        np.tile(np.eye(64, dtype=np.float32), (2, 2))
    ).astype(NP_BF16)                            # [128, 128]
    brhs = np.ascontiguousarray(
        np.tile(bias.reshape(64, 64), (2, 8))
    ).astype(NP_BF16)                            # [128, 512]

    in_maps = []
    xr = x.reshape(B_FULL, 64, 64)               # [b, i12, i34]
    for c in range(N_CORES):
        xc = xr[c * B_L : (c + 1) * B_L]         # [512, 64, 64]
        # t[pair, i34, bhat*64+i12]
        t = xc.reshape(NPAIR, 2, 64, 64).transpose(0, 3, 1, 2).reshape(
            NPAIR, 64, 128
        )
        # xpp[(codd*64 + i34), blk*128 + m] = t[blk*2 + codd, i34, m]
        t2 = t.reshape(NPAIR // 2, 2, 64, 128)
        xpp = np.ascontiguousarray(
            t2.transpose(1, 2, 0, 3).reshape(128, NPAIR * 64)
        ).astype(NP_BF16)
        in_maps.append(
            {
                "xpp": xpp,
                "bmat": bm,
                "atil": at,
                "eye": eyev,
                "brhs": brhs,
            }
        )
    return in_maps


def kernel(x, core0, core1, core2, core3, b) -> np.ndarray:
    x = np.asarray(x, dtype=np.float32)
    A, B = _merge_cores(
        np.asarray(core0, dtype=np.float32),
        np.asarray(core1, dtype=np.float32),
        np.asarray(core2, dtype=np.float32),
        np.asarray(core3, dtype=np.float32),
    )
    bias = np.asarray(b, dtype=np.float32)

    if "nc" not in _CACHE:
        _CACHE["nc"] = _build_module()
    nc = _CACHE["nc"]

    in_maps = _prepare_in_maps(x, A, B, bias)
    res = run_bass_kernel_spmd(nc, in_maps, core_ids=list(range(N_CORES)))

    y = np.empty((B_FULL, O_FULL), dtype=np.float32)
    for c in range(N_CORES):
        arr = np.asarray(res.results[c]["y"]).astype(np.float32)
        # [g, bhat, j12, s, j34] -> b_local = 16g + 2s + bhat
        t = arr.reshape(32, 2, 64, 8, 64).transpose(0, 3, 1, 2, 4)
        y[c * B_L : (c + 1) * B_L] = t.reshape(B_L, O_FULL)
    return y


# revision 3
# speedup vs baseline: 1.0126x; 1.0118x over previous
"""Trainium2 TT-structured kernel for nn_KerasDense_32263794328408.

y = relu(x @ M + b), M = TT-matrix, ranks [1,8,8,8,1], modes 8x8x8x8.

Algebra: merging cores (0,1) -> A[i12, j12, r] (64x64x8) and cores
(2,3) -> B[r, i34, j34] (8x64x64) gives M = sum_r A_r (x) B_r (a
rank-8 Kronecker sum). Contracting B first then A needs 34.4 GFLOP
total vs 137.4 GFLOP dense -- 4x fewer. ~125 us vs the 237 us dense
near-roofline GEMM this replaced (1.9x).

Two-stage PE schedule per core (batch-sharded 8 ways, B_L=512):

Stage 1 (contract i34, K=64): z[b, i12, r, j34] = sum_i34 x[b,i12,i34]
  * B[r,i34,j34]. Stationary = x'' chunk [64 i34, 128 (b-hat,i12)]
  (one chunk = 2 batch rows x 64 i12), moving = Bmat [64, 512 (r,j34)].
  K=64 would waste half the array, so two chunks run CONCURRENTLY via
  tile_position row-tiling (rows 0-63 / 64-127, separate PSUM banks;
  measured 3ns apart). 256 chunks -> 128 windows at full PE rate.

Stage 2 (contract (i12, r), K=512): y[b, j12, j34] = sum A z. r rides
  the PSUM accumulation loop (8 matmuls per bank), i12 rides the
  partitions: stationary At_r = diag(A_r, A_r) block-diagonal so the
  two batch rows of a chunk don't mix. Each r-matmul streams all 8
  pairs of a bank in one N=512 pass via a 3D strided rhs [128, s=8,
  64] into the shared z-bank tile. Bias = one K=64 matmul (duplicated
  identity stationary x bias rows) opening each bank's group; its
  [64,128] stationary shape matches the windows so it rides in their
  LDWEIGHTS stream.

z (16.8M elem/core) must cross PSUM->SBUF through compute engines
(DMA has no PSUM port, GPSIMD no PSUM access, PSUM reads force DVE
1x mode): one fused [128,1024] copy per window, alternating ACT/DVE,
with the relu+bf16 cast of finished y banks 2:1 on ACT/DVE. Both
engines run ~83% busy under the PE.

Schedule: units of [winA winB bias][r0-r3][winC winD][r4-r7+relu],
step2 lagging its z-bank by 2 units. This grouping holds the PE
shape-transition count at 4/unit (each win<->step2 switch exposes
~100ns of LDWEIGHTS); 3-window bursts overrun the 3-deep [128,1024]
PSUM pool (drains pace at ~1.2us/window). Steady state: 3227 ns/unit,
PE fully dense. Warmup: 40 HAM-lift matmuls on a memset tile (no DMA
dependency) bridge the ~7-11us preamble so the clock gate opens
before the first real matmul; first xpp segment is split 4-ways so
window 0's data lands early.

y is stored bank-contiguous ([128, 512] blocks, fully-contiguous
128 KB store DMAs on the sync ring) and un-permuted on the host.

Everything ships bf16; accumulation is fp32 in PSUM. absmax/scale =
4.4e-3 (gate 2e-2); fp8 anywhere pushes past the gate (z or step2 in
e4m3 -> ~5% relative error over the K=512 contraction).
"""

import sys

if "/opt/trn_rl_repo" not in sys.path:
    sys.path.insert(0, "/opt/trn_rl_repo")

import ml_dtypes
import numpy as np

import concourse.bacc as bacc
import concourse.bass as bass
import concourse.mybir as mybir
import concourse.tile as tile
from concourse.bass_utils import run_bass_kernel_spmd

F32 = mybir.dt.float32
BF16 = mybir.dt.bfloat16
NP_BF16 = ml_dtypes.bfloat16

B_FULL = 4096
F_FULL = 4096
O_FULL = 4096
N_CORES = 8
B_L = B_FULL // N_CORES          # 512 batch rows per core

NPAIR = B_L // 2                 # 256 chunks (= batch pairs) per core
NBCH = 8                         # bchunks
PAIR_PER_BCH = NPAIR // NBCH     # 32
NWIN = PAIR_PER_BCH // 2         # 16 step1 windows per bchunk
NBANK = PAIR_PER_BCH // 8        # 4 step2 banks per bchunk
R = 8

_CACHE: dict = {}


def _build_module() -> bass.Bass:
    nc = bacc.Bacc(None, target_bir_lowering=False)

    # x'': row-tiled stationary chunks. Column block w = 128 cols holds
    # chunks 2w (partitions 0-63) and 2w+1 (partitions 64-127); within a
    # block, col m = bhat*64 + i12, partition (c%2)*64 + i34.
    xpp = nc.declare_dram_parameter("xpp", [128, NPAIR * 64], BF16, isOutput=False)
    # Bmat moving operand, both partition halves identical:
    # bmat[h*64 + i34, r*64 + j34] = B[r, i34, j34]
    bmat = nc.declare_dram_parameter("bmat", [128, 512], BF16, isOutput=False)
    # At_r block-diagonal stationaries, concat along free dim.
    atil = nc.declare_dram_parameter("atil", [128, R * 128], BF16, isOutput=False)
    # Bias: one K=64 matmul per bank. eye[j12p, bhat*64+j12]=delta,
    # brhs[j12p, s*64+j34] = bias[j12p*64+j34].
    eye = nc.declare_dram_parameter("eye", [128, 128], BF16, isOutput=False)
    brhs = nc.declare_dram_parameter("brhs", [128, 512], BF16, isOutput=False)
    # y stored bank-major: row g*128 + (bhat*64 + j12), col s*64 + j34;
    # host un-permutes. Keeps every store DMA fully contiguous.
    y = nc.declare_dram_parameter("y", [32 * 128, 512], BF16, isOutput=True)

    with tile.TileContext(nc) as tc:
        with (
            tc.tile_pool(name="xt", bufs=1) as x_pool,
            tc.tile_pool(name="cst", bufs=1) as c_pool,
            tc.tile_pool(name="z1", bufs=1) as z_pool,
            tc.tile_pool(name="ysb", bufs=1) as y_pool,
            tc.tile_pool(name="ps1", bufs=3, space="PSUM") as ps1_pool,
            tc.tile_pool(name="ps2", bufs=2, space="PSUM") as ps2_pool,
        ):
            # Warmup operand via memset: no DMA dependency, so the HAM
            # warmup matmuls start right after engine init (~4us) instead
            # of waiting for the first DMA (~10us).
            ones_sb = c_pool.tile([128, 128], BF16, tag="ones")
            nc.gpsimd.memset(ones_sb[:], 1.0)
            bmat_sb = c_pool.tile([128, 512], BF16, tag="bmat")
            nc.sync.dma_start(out=bmat_sb[:], in_=bmat[:])

            # x'' in 8 bchunk pieces; first two segments ahead of the
            # step2 constants (not needed until unit 2).
            xpp_sb = x_pool.tile([128, NPAIR * 64], BF16, tag="xpp")
            seg = PAIR_PER_BCH * 64
            sub = seg // 4
            for j in range(4):
                nc.sync.dma_start(
                    out=xpp_sb[:, j * sub : (j + 1) * sub],
                    in_=xpp[:, j * sub : (j + 1) * sub],
                )
            nc.sync.dma_start(
                out=xpp_sb[:, seg : 2 * seg], in_=xpp[:, seg : 2 * seg]
            )
            eye_sb = c_pool.tile([128, 128], BF16, tag="eye")
            nc.sync.dma_start(out=eye_sb[:], in_=eye[:])
            brhs_sb = c_pool.tile([128, 512], BF16, tag="brhs")
            nc.sync.dma_start(out=brhs_sb[:], in_=brhs[:])
            atil_sb = c_pool.tile([128, R * 128], BF16, tag="atil")
            nc.sync.dma_start(out=atil_sb[:], in_=atil[:])
            for k in range(2, NBCH):
                nc.sync.dma_start(
                    out=xpp_sb[:, k * seg : (k + 1) * seg],
                    in_=xpp[:, k * seg : (k + 1) * seg],
                )

            # HAM warmup: full-K dummy matmuls while the first loads land.
            warm = ps2_pool.tile([128, 512], F32, tag="ps2", name="warm")
            for i in range(40):
                nc.tensor.matmul(
                    warm[:, 0:128],
                    ones_sb[:],
                    ones_sb[:],
                    start=(i == 0),
                    stop=(i == 39),
                )

            ZB = 4  # z_bank tiles in flight (write g, drain g/g-1, read g-2)
            z_banks = [None] * ZB

            def step1_window(w):
                # window w: chunks 2w (rows 0-63), 2w+1 (rows 64-127),
                # both -> one [128, 1024] psum pair, one fused drain into
                # z_bank[w//4] cols (2w%8)*512 .. +1024.
                g = w // 4
                if w % 4 == 0:
                    zb = z_pool.tile(
                        [128, 8 * 512], BF16, tag=f"zb{g % ZB}",
                        name=f"zb_{g}"
                    )
                    z_banks[g % ZB] = zb
                zb = z_banks[g % ZB]
                ps = ps1_pool.tile([128, 1024], F32, tag="ps1",
                                   name=f"z_{w}")
                for half in (0, 1):
                    nc.tensor.matmul(
                        ps[:, half * 512 : half * 512 + 512],
                        xpp_sb[half * 64 : half * 64 + 64,
                               w * 128 : (w + 1) * 128],
                        bmat_sb[half * 64 : half * 64 + 64, :],
                        start=True,
                        stop=True,
                        tile_position=(half * 64, 0),
                    )
                dst = zb[:, (2 * w % 8) * 512 : (2 * w % 8) * 512 + 1024]
                if w % 2 == 0:
                    nc.scalar.copy(dst, ps[:])
                else:
                    nc.vector.tensor_copy(dst, ps[:])

            y_ps = {}

            def step2_piece(g, piece):
                # bank g's PE work, split into 5 pieces interleaved between
                # step1 windows: [bias], [r0 r1], [r2 r3], [r4 r5], [r6 r7
                # + relu + store].
                zb = z_banks[g % ZB]
                if piece == 0:
                    ps = ps2_pool.tile([128, 512], F32, tag="ps2",
                                       name=f"y_{g}")
                    y_ps[g % 2] = ps
                    nc.tensor.matmul(
                        ps[:], eye_sb[0:64, :], brhs_sb[0:64, :],
                        start=True, stop=False,
                        skip_group_check=True,
                    )
                    return
                ps = y_ps[g % 2]
                z3 = zb[:].rearrange("p (s q) -> p s q", q=512)
                for r in (2 * piece - 2, 2 * piece - 1):
                    nc.tensor.matmul(
                        ps[:],
                        atil_sb[:, r * 128 : (r + 1) * 128],
                        z3[:, :, r * 64 : (r + 1) * 64],
                        start=False,
                        stop=(r == R - 1),
                        skip_group_check=True,
                    )
                if piece == 4:
                    ysb = y_pool.tile(
                        [128, 512], BF16, tag=f"y{g % 4}", name=f"ysb_{g}"
                    )
                    if g % 3 != 0:
                        nc.scalar.activation(
                            ysb[:], ps[:], mybir.ActivationFunctionType.Relu
                        )
                    else:
                        nc.vector.tensor_scalar_max(ysb[:], ps[:], 0.0)
                    nc.sync.dma_start(
                        out=y[g * 128 : (g + 1) * 128, :], in_=ysb[:]
                    )

            NG = NPAIR // 8  # 32 banks
            # Unit u: windows 4u..4u+3 + step2 of bank u-2, grouped to
            # minimize PE shape transitions (each win<->step2 switch costs
            # ~100ns of exposed LDWEIGHTS): [winA winB bias][r0-r3]
            # [winC winD][r4-r7 relu].
            for u in range(NG + 2):
                if u < NG:
                    step1_window(4 * u)
                    step1_window(4 * u + 1)
                if u >= 2:
                    step2_piece(u - 2, 0)   # bias
                    step2_piece(u - 2, 1)   # r0 r1
                    step2_piece(u - 2, 2)   # r2 r3
                if u < NG:
                    step1_window(4 * u + 2)
                    step1_window(4 * u + 3)
                if u >= 2:
                    step2_piece(u - 2, 3)   # r4 r5
                    step2_piece(u - 2, 4)   # r6 r7 + relu + store

    nc.finalize()
    return nc


def _merge_cores(core0, core1, core2, core3):
    g1 = core0[0].astype(np.float64)            # [m1, n1, r1]
    t12 = np.einsum("mnr,rMNs->mMnNs", g1, core1.astype(np.float64))
    A = t12.reshape(64, 64, 8)                  # [i12, j12, r]
    g4 = core3[..., 0].astype(np.float64)       # [r3, m4, n4]
    t34 = np.einsum("rmns,sMN->rmMnN", core2.astype(np.float64), g4)
    B = t34.reshape(8, 64, 64)                  # [r, i34, j34]
    return A.astype(np.float32), B.astype(np.float32)


def _prepare_in_maps(x, A, B, bias):
    # Shared constants
    bm = np.ascontiguousarray(
        np.tile(B.transpose(1, 0, 2).reshape(64, 512), (2, 1))
    ).astype(NP_BF16)                            # [128, 512]
    at = np.zeros((R, 128, 128), dtype=np.float32)
    for r in range(R):
        at[r, 0:64, 0:64] = A[:, :, r]
        at[r, 64:128, 64:128] = A[:, :, r]
    at = np.ascontiguousarray(
        at.transpose(1, 0, 2).reshape(128, R * 128)
    ).astype(NP_BF16)
    eyev = np.ascontiguousarray(
        np.tile(np.eye(64, dtype=np.float32), (2, 2))
    ).astype(NP_BF16)                            # [128, 128]
    brhs = np.ascontiguousarray(
        np.tile(bias.reshape(64, 64), (2, 8))
    ).astype(NP_BF16)                            # [128, 512]

    in_maps = []
    xr = x.reshape(B_FULL, 64, 64)               # [b, i12, i34]
    for c in range(N_CORES):
        xc = xr[c * B_L : (c + 1) * B_L]         # [512, 64, 64]
        # t[pair, i34, bhat*64+i12]
        t = xc.reshape(NPAIR, 2, 64, 64).transpose(0, 3, 1, 2).reshape(
            NPAIR, 64, 128
        )
        # xpp[(codd*64 + i34), blk*128 + m] = t[blk*2 + codd, i34, m]
        t2 = t.reshape(NPAIR // 2, 2, 64, 128)
        xpp = np.ascontiguousarray(
            t2.transpose(1, 2, 0, 3).reshape(128, NPAIR * 64)
        ).astype(NP_BF16)
        in_maps.append(
            {
                "xpp": xpp,
                "bmat": bm,
                "atil": at,
                "eye": eyev,
                "brhs": brhs,
            }
        )
    return in_maps


def kernel(x, core0, core1, core2, core3, b) -> np.ndarray:
    x = np.asarray(x, dtype=np.float32)
    A, B = _merge_cores(
        np.asarray(core0, dtype=np.float32),
        np.asarray(core1, dtype=np.float32),
        np.asarray(core2, dtype=np.float32),
        np.asarray(core3, dtype=np.float32),
    )
    bias = np.asarray(b, dtype=np.float32)

    if "nc" not in _CACHE:
        _CACHE["nc"] = _build_module()
    nc = _CACHE["nc"]

    in_maps = _prepare_in_maps(x, A, B, bias)
    res = run_bass_kernel_spmd(nc, in_maps, core_ids=list(range(N_CORES)))

    y = np.empty((B_FULL, O_FULL), dtype=np.float32)
    for c in range(N_CORES):
        arr = np.asarray(res.results[c]["y"]).astype(np.float32)
        # [g, bhat, j12, s, j34] -> b_local = 16g + 2s + bhat
        t = arr.reshape(32, 2, 64, 8, 64).transpose(0, 3, 1, 2, 4)
        y[c * B_L : (c + 1) * B_L] = t.reshape(B_L, O_FULL)
    return y


# revision 4
# speedup vs baseline: 1.0153x; 1.0027x over previous
"""Trainium2 TT-structured kernel for nn_KerasDense_32263794328408.

y = relu(x @ M + b), M = TT-matrix, ranks [1,8,8,8,1], modes 8x8x8x8.

Algebra: merging cores (0,1) -> A[i12, j12, r] (64x64x8) and cores
(2,3) -> B[r, i34, j34] (8x64x64) gives M = sum_r A_r (x) B_r (a
rank-8 Kronecker sum). Contracting B first then A needs 34.4 GFLOP
total vs 137.4 GFLOP dense -- 4x fewer. ~125 us vs the 237 us dense
near-roofline GEMM this replaced (1.9x).

Two-stage PE schedule per core (batch-sharded 8 ways, B_L=512):

Stage 1 (contract i34, K=64): z[b, i12, r, j34] = sum_i34 x[b,i12,i34]
  * B[r,i34,j34]. Stationary = x'' chunk [64 i34, 128 (b-hat,i12)]
  (one chunk = 2 batch rows x 64 i12), moving = Bmat [64, 512 (r,j34)].
  K=64 would waste half the array, so two chunks run CONCURRENTLY via
  tile_position row-tiling (rows 0-63 / 64-127, separate PSUM banks;
  measured 3ns apart). 256 chunks -> 128 windows at full PE rate.

Stage 2 (contract (i12, r), K=512): y[b, j12, j34] = sum A z. r rides
  the PSUM accumulation loop (8 matmuls per bank), i12 rides the
  partitions: stationary At_r = diag(A_r, A_r) block-diagonal so the
  two batch rows of a chunk don't mix. Each r-matmul streams all 8
  pairs of a bank in one N=512 pass via a 3D strided rhs [128, s=8,
  64] into the shared z-bank tile. Bias = one K=64 matmul (duplicated
  identity stationary x bias rows) opening each bank's group; its
  [64,128] stationary shape matches the windows so it rides in their
  LDWEIGHTS stream.

z (16.8M elem/core) must cross PSUM->SBUF through compute engines
(DMA has no PSUM port, GPSIMD no PSUM access, PSUM reads force DVE
1x mode): one fused [128,1024] copy per window, alternating ACT/DVE,
with the relu+bf16 cast of finished y banks 2:1 on ACT/DVE. Both
engines run ~83% busy under the PE.

Schedule: units of [winA winB bias][r0-r3][winC winD][r4-r7+relu],
step2 lagging its z-bank by 2 units. This grouping holds the PE
shape-transition count at 4/unit (each win<->step2 switch exposes
~100ns of LDWEIGHTS); 3-window bursts overrun the 3-deep [128,1024]
PSUM pool (drains pace at ~1.2us/window). Steady state: 3227 ns/unit,
PE fully dense. Warmup: 40 HAM-lift matmuls on a memset tile (no DMA
dependency) bridge the ~7-11us preamble so the clock gate opens
before the first real matmul; first xpp segment is split 4-ways so
window 0's data lands early.

y is stored bank-contiguous ([128, 512] blocks, fully-contiguous
128 KB store DMAs on the sync ring) and un-permuted on the host.

Everything ships bf16; accumulation is fp32 in PSUM. absmax/scale =
4.4e-3 (gate 2e-2); fp8 anywhere pushes past the gate (z or step2 in
e4m3 -> ~5% relative error over the K=512 contraction).
"""

import sys

if "/opt/trn_rl_repo" not in sys.path:
    sys.path.insert(0, "/opt/trn_rl_repo")

import ml_dtypes
import numpy as np

import concourse.bacc as bacc
import concourse.bass as bass
import concourse.mybir as mybir
import concourse.tile as tile
from concourse.bass_utils import run_bass_kernel_spmd

F32 = mybir.dt.float32
BF16 = mybir.dt.bfloat16
NP_BF16 = ml_dtypes.bfloat16

B_FULL = 4096
F_FULL = 4096
O_FULL = 4096
N_CORES = 8
B_L = B_FULL // N_CORES          # 512 batch rows per core

NPAIR = B_L // 2                 # 256 chunks (= batch pairs) per core
NBCH = 8                         # bchunks
PAIR_PER_BCH = NPAIR // NBCH     # 32
NWIN = PAIR_PER_BCH // 2         # 16 step1 windows per bchunk
NBANK = PAIR_PER_BCH // 8        # 4 step2 banks per bchunk
R = 8

_CACHE: dict = {}


def _build_module() -> bass.Bass:
    nc = bacc.Bacc(None, target_bir_lowering=False)

    # x'': row-tiled stationary chunks. Column block w = 128 cols holds
    # chunks 2w (partitions 0-63) and 2w+1 (partitions 64-127); within a
    # block, col m = bhat*64 + i12, partition (c%2)*64 + i34.
    xpp = nc.declare_dram_parameter("xpp", [128, NPAIR * 64], BF16, isOutput=False)
    # Bmat moving operand, both partition halves identical:
    # bmat[h*64 + i34, r*64 + j34] = B[r, i34, j34]
    bmat = nc.declare_dram_parameter("bmat", [128, 512], BF16, isOutput=False)
    # At_r block-diagonal stationaries, concat along free dim.
    atil = nc.declare_dram_parameter("atil", [128, R * 128], BF16, isOutput=False)
    # Bias: one K=64 matmul per bank. eye[j12p, bhat*64+j12]=delta,
    # brhs[j12p, s*64+j34] = bias[j12p*64+j34].
    eye = nc.declare_dram_parameter("eye", [128, 128], BF16, isOutput=False)
    brhs = nc.declare_dram_parameter("brhs", [128, 512], BF16, isOutput=False)
    # y stored bank-major: row g*128 + (bhat*64 + j12), col s*64 + j34;
    # host un-permutes. Keeps every store DMA fully contiguous.
    y = nc.declare_dram_parameter("y", [32 * 128, 512], BF16, isOutput=True)

    with tile.TileContext(nc) as tc:
        with (
            tc.tile_pool(name="xt", bufs=1) as x_pool,
            tc.tile_pool(name="cst", bufs=1) as c_pool,
            tc.tile_pool(name="z1", bufs=1) as z_pool,
            tc.tile_pool(name="ysb", bufs=1) as y_pool,
            tc.tile_pool(name="ps1", bufs=3, space="PSUM") as ps1_pool,
            tc.tile_pool(name="ps2", bufs=2, space="PSUM") as ps2_pool,
        ):
            # Warmup operand via memset: no DMA dependency, so the HAM
            # warmup matmuls start right after engine init (~4us) instead
            # of waiting for the first DMA (~10us).
            ones_sb = c_pool.tile([128, 128], BF16, tag="ones")
            nc.gpsimd.memset(ones_sb[:], 1.0)
            bmat_sb = c_pool.tile([128, 512], BF16, tag="bmat")
            nc.sync.dma_start(out=bmat_sb[:], in_=bmat[:])

            # x'' in 8 bchunk pieces; first two segments ahead of the
            # step2 constants (not needed until unit 2).
            xpp_sb = x_pool.tile([128, NPAIR * 64], BF16, tag="xpp")
            seg = PAIR_PER_BCH * 64
            sub = seg // 4
            for j in range(4):
                nc.sync.dma_start(
                    out=xpp_sb[:, j * sub : (j + 1) * sub],
                    in_=xpp[:, j * sub : (j + 1) * sub],
                )
            nc.sync.dma_start(
                out=xpp_sb[:, seg : 2 * seg], in_=xpp[:, seg : 2 * seg]
            )
            eye_sb = c_pool.tile([128, 128], BF16, tag="eye")
            nc.sync.dma_start(out=eye_sb[:], in_=eye[:])
            brhs_sb = c_pool.tile([128, 512], BF16, tag="brhs")
            nc.sync.dma_start(out=brhs_sb[:], in_=brhs[:])
            atil_sb = c_pool.tile([128, R * 128], BF16, tag="atil")
            nc.sync.dma_start(out=atil_sb[:], in_=atil[:])
            for k in range(2, NBCH):
                nc.sync.dma_start(
                    out=xpp_sb[:, k * seg : (k + 1) * seg],
                    in_=xpp[:, k * seg : (k + 1) * seg],
                )

            # HAM warmup: full-K dummy matmuls while the first loads land.
            warm = ps2_pool.tile([128, 512], F32, tag="ps2", name="warm")
            for i in range(40):
                nc.tensor.matmul(
                    warm[:, 0:128],
                    ones_sb[:],
                    ones_sb[:],
                    start=(i == 0),
                    stop=(i == 39),
                )

            ZB = 4  # z_bank tiles in flight (write g, drain g/g-1, read g-2)
            z_banks = [None] * ZB

            def step1_window(w):
                # window w: chunks 2w (rows 0-63), 2w+1 (rows 64-127),
                # both -> one [128, 1024] psum pair, one fused drain into
                # z_bank[w//4] cols (2w%8)*512 .. +1024.
                g = w // 4
                if w % 4 == 0:
                    zb = z_pool.tile(
                        [128, 8 * 512], BF16, tag=f"zb{g % ZB}",
                        name=f"zb_{g}"
                    )
                    z_banks[g % ZB] = zb
                zb = z_banks[g % ZB]
                ps = ps1_pool.tile([128, 1024], F32, tag="ps1",
                                   name=f"z_{w}")
                for half in (0, 1):
                    nc.tensor.matmul(
                        ps[:, half * 512 : half * 512 + 512],
                        xpp_sb[half * 64 : half * 64 + 64,
                               w * 128 : (w + 1) * 128],
                        bmat_sb[half * 64 : half * 64 + 64, :],
                        start=True,
                        stop=True,
                        tile_position=(half * 64, 0),
                    )
                dst = zb[:, (2 * w % 8) * 512 : (2 * w % 8) * 512 + 1024]
                if w % 2 == 0:
                    nc.scalar.copy(dst, ps[:])
                else:
                    nc.vector.tensor_copy(dst, ps[:])

            y_ps = {}

            def step2_piece(g, piece):
                # bank g's PE work, split into 5 pieces interleaved between
                # step1 windows: [bias], [r0 r1], [r2 r3], [r4 r5], [r6 r7
                # + relu + store].
                zb = z_banks[g % ZB]
                if piece == 0:
                    ps = ps2_pool.tile([128, 512], F32, tag="ps2",
                                       name=f"y_{g}")
                    y_ps[g % 2] = ps
                    nc.tensor.matmul(
                        ps[:], eye_sb[0:64, :], brhs_sb[0:64, :],
                        start=True, stop=False,
                        skip_group_check=True,
                    )
                    return
                ps = y_ps[g % 2]
                z3 = zb[:].rearrange("p (s q) -> p s q", q=512)
                for r in (2 * piece - 2, 2 * piece - 1):
                    nc.tensor.matmul(
                        ps[:],
                        atil_sb[:, r * 128 : (r + 1) * 128],
                        z3[:, :, r * 64 : (r + 1) * 64],
                        start=False,
                        stop=(r == R - 1),
                        skip_group_check=True,
                    )
                if piece == 4:
                    ysb = y_pool.tile(
                        [128, 512], BF16, tag=f"y{g % 4}", name=f"ysb_{g}"
                    )
                    if g % 3 != 0:
                        nc.scalar.activation(
                            ysb[:], ps[:], mybir.ActivationFunctionType.Relu
                        )
                    else:
                        nc.vector.tensor_scalar_max(ysb[:], ps[:], 0.0)
                    nc.sync.dma_start(
                        out=y[g * 128 : (g + 1) * 128, :], in_=ysb[:]
                    )

            NG = NPAIR // 8  # 32 banks
            # Unit u: windows 4u..4u+3 + step2 of bank u-2, grouped to
            # minimize PE shape transitions (each win<->step2 switch costs
            # ~100ns of exposed LDWEIGHTS): [winA winB bias][r0-r3]
            # [winC winD][r4-r7 relu].
            def warm_fill(n):
                # Ramp filler: units 0-1 are drain-paced and the PE idles
                # between windows; without these the HAM MID window can
                # re-throttle the clock to 1.2 GHz right as the steady
                # stream begins.
                for i in range(n):
                    nc.tensor.matmul(
                        warm[:, 0:128], ones_sb[:], ones_sb[:],
                        start=(i == 0), stop=(i == n - 1),
                    )

            for u in range(NG + 2):
                if u < NG:
                    step1_window(4 * u)
                    step1_window(4 * u + 1)
                if u < 2:
                    warm_fill(6)
                if u >= 2:
                    step2_piece(u - 2, 0)   # bias
                    step2_piece(u - 2, 1)   # r0 r1
                    step2_piece(u - 2, 2)   # r2 r3
                if u < NG:
                    step1_window(4 * u + 2)
                    step1_window(4 * u + 3)
                if u < 2:
                    warm_fill(6)
                if u >= 2:
                    step2_piece(u - 2, 3)   # r4 r5
                    step2_piece(u - 2, 4)   # r6 r7 + relu + store

    nc.finalize()
    return nc


def _merge_cores(core0, core1, core2, core3):
    g1 = core0[0].astype(np.float64)            # [m1, n1, r1]
    t12 = np.einsum("mnr,rMNs->mMnNs", g1, core1.astype(np.float64))
    A = t12.reshape(64, 64, 8)                  # [i12, j12, r]
    g4 = core3[..., 0].astype(np.float64)       # [r3, m4, n4]
    t34 = np.einsum("rmns,sMN->rmMnN", core2.astype(np.float64), g4)
    B = t34.reshape(8, 64, 64)                  # [r, i34, j34]
    return A.astype(np.float32), B.astype(np.float32)


def _prepare_in_maps(x, A, B, bias):
    # Shared constants
    bm = np.ascontiguousarray(
        np.tile(B.transpose(1, 0, 2).reshape(64, 512), (2, 1))
    ).astype(NP_BF16)                            # [128, 512]
    at = np.zeros((R, 128, 128), dtype=np.float32)
    for r in range(R):
        at[r, 0:64, 0:64] = A[:, :, r]
        at[r, 64:128, 64:128] = A[:, :, r]
    at = np.ascontiguousarray(
        at.transpose(1, 0, 2).reshape(128, R * 128)
    ).astype(NP_BF16)
    eyev = np.ascontiguousarray(
        np.tile(np.eye(64, dtype=np.float32), (2, 2))
    ).astype(NP_BF16)                            # [128, 128]
    brhs = np.ascontiguousarray(
        np.tile(bias.reshape(64, 64), (2, 8))
    ).astype(NP_BF16)                            # [128, 512]

    in_maps = []
    xr = x.reshape(B_FULL, 64, 64)               # [b, i12, i34]
    for c in range(N_CORES):
        xc = xr[c * B_L : (c + 1) * B_L]         # [512, 64, 64]
        # t[pair, i34, bhat*64+i12]
        t = xc.reshape(NPAIR, 2, 64, 64).transpose(0, 3, 1, 2).reshape(
            NPAIR, 64, 128
        )
        # xpp[(codd*64 + i34), blk*128 + m] = t[blk*2 + codd, i34, m]
        t2 = t.reshape(NPAIR // 2, 2, 64, 128)
        xpp = np.ascontiguousarray(
            t2.transpose(1, 2, 0, 3).reshape(128, NPAIR * 64)
        ).astype(NP_BF16)
        in_maps.append(
            {
                "xpp": xpp,
                "bmat": bm,
                "atil": at,
                "eye": eyev,
                "brhs": brhs,
            }
        )
    return in_maps


def kernel(x, core0, core1, core2, core3, b) -> np.ndarray:
    x = np.asarray(x, dtype=np.float32)
    A, B = _merge_cores(
        np.asarray(core0, dtype=np.float32),
        np.asarray(core1, dtype=np.float32),
        np.asarray(core2, dtype=np.float32),
        np.asarray(core3, dtype=np.float32),
    )
    bias = np.asarray(b, dtype=np.float32)

    if "nc" not in _CACHE:
        _CACHE["nc"] = _build_module()
    nc = _CACHE["nc"]

    in_maps = _prepare_in_maps(x, A, B, bias)
    res = run_bass_kernel_spmd(nc, in_maps, core_ids=list(range(N_CORES)))

    y = np.empty((B_FULL, O_FULL), dtype=np.float32)
    for c in range(N_CORES):
        arr = np.asarray(res.results[c]["y"]).astype(np.float32)
        # [g, bhat, j12, s, j34] -> b_local = 16g + 2s + bhat
        t = arr.reshape(32, 2, 64, 8, 64).transpose(0, 3, 1, 2, 4)
        y[c * B_L : (c + 1) * B_L] = t.reshape(B_L, O_FULL)
    return y
